# revision 1
# baseline (speedup 1.0000x reference)
"""Dual-stream attention (nn_Attention2) on 8 TRN2 NeuronCores, v2.

Problem: B=4, N=1024, C=768, H=12, D=64.
  qkv_s = x_s @ W_qkv.T + b_qkv          (s = 1,2; shared weights)
  attn  = softmax(q1k1/sqrt(D) + q2k2/sqrt(D))   (one shared softmax)
  o_s   = attn @ v_s;  y_s = o_s @ W_proj.T + b_proj

Sharding: 8 cores = 4 batches x 2 head-groups (6 heads each), as v1, but
host<->device traffic is minimized with on-device collectives:
  * x upload halved: core (b,g) uploads only token-half g of x1t/x2t
    (bf16); a pair AllGather rebuilds the full [C,N] per batch on device.
  * weight upload quartered: the per-head-group weight blob [1536,768]
    (wqk | wv | wproj) is AllGathered from per-core quarters across the
    4 batch-cores of each head-group ({0,2,4,6} / {1,3,5,7}).
  * y download quartered: per-core partial yT (both streams, stacked
    [2C,N] bf16) is pair-ReduceScattered so core (b,0) outputs the final
    y1T(b) and core (b,1) the final y2T(b), bf16. b_proj/2 is added on
    device by each pair member, so the host only transposes/casts.
Compute phases are v1's: stream-stacked q/k tiles ([128,N] =
[s1 64 | s2 64]) make combined scores one matmul chain; attention runs
in sT[k,q] orientation with an all-ones matmul producing the softmax
denominator broadcast across partitions; normalization applies to oT
before the projection. bf16 matmuls, f32 PSUM.
"""

import contextlib
import threading

import numpy as np
import ml_dtypes
import jax
from jax.sharding import Mesh, PartitionSpec
try:
    from jax.experimental.shard_map import shard_map
except ImportError:
    from jax.sharding import shard_map

import concourse.bass as bass
import concourse.tile as tile
from concourse import bacc, mybir
from concourse.bass_utils import run_bass_kernel_spmd
from concourse.bass2jax import (
    install_neuronx_cc_hook,
    partition_id_tensor,
    _bass_exec_p,
)

F32 = mybir.dt.float32
BF16 = mybir.dt.bfloat16
AL = mybir.AluOpType
AF = mybir.ActivationFunctionType

B, N, C, H = 4, 1024, 768, 12
D = C // H              # 64
HPC = 6                 # heads per core
KT = C // 128           # 6 contraction tiles over C
NQ = N // 512           # 2 q-halves
NK = N // 128            # 8 k-blocks
SCALE = float(D) ** -0.5
BF = ml_dtypes.bfloat16

PAIRS = [[0, 1], [2, 3], [4, 5], [6, 7]]
QUADS = [[0, 2, 4, 6], [1, 3, 5, 7]]


def build_program(loop_reps=0, phase_cut=None, use_cc=True):
    """use_cc=False replaces collectives with local DMAs moving the same
    bytes (collectives desync the mesh inside a For_i hardware loop, so
    timing builds approximate them; the graded single-shot path keeps
    real collectives)."""
    nc = bacc.Bacc("TRN2", target_bir_lowering=False, debug=False)

    xh = nc.dram_tensor("xh", [2 * C, 512], BF16, kind="ExternalInput").ap()
    wblob = nc.dram_tensor("wblob", [384, C], BF16, kind="ExternalInput").ap()
    bqkv = nc.dram_tensor("bqkv", [128, 2 * HPC], F32, kind="ExternalInput").ap()
    bv = nc.dram_tensor("bv", [1, HPC * D], F32, kind="ExternalInput").ap()
    bproj = nc.dram_tensor("bproj", [128, KT], F32, kind="ExternalInput").ap()
    yt = nc.dram_tensor("yt", [C, N], BF16, kind="ExternalOutput").ap()

    with tile.TileContext(nc) as tc:
        with (
            tc.tile_pool(name="dram", bufs=1, space="DRAM") as dp_,
            tc.tile_pool(name="persist", bufs=1) as pp,
            tc.tile_pool(name="expp", bufs=5) as ep,
            tc.tile_pool(name="rdp", bufs=3) as rp,
            tc.tile_pool(name="ybp", bufs=4) as yp,
            tc.For_i(0, loop_reps, 1) if loop_reps else contextlib.nullcontext(),
        ):
            # ---- collective staging: gather weights (quads) and x (pairs) ----
            wb_b = dp_.tile([384, C], BF16, tag="wb_b")
            wg = dp_.tile([4 * 384, C], BF16, tag="wg")
            xh_b = dp_.tile([2 * C, 512], BF16, tag="xh_b")
            xg = dp_.tile([4 * C, 512], BF16, tag="xg")
            nc.sync.dma_start(out=wb_b[:], in_=wblob)
            if use_cc:
                nc.gpsimd.collective_compute(
                    "AllGather", AL.bypass, replica_groups=QUADS,
                    ins=[wb_b[:].opt()], outs=[wg[:].opt()])
            else:
                for r in range(4):
                    nc.gpsimd.dma_start(
                        out=wg[r * 384:(r + 1) * 384, :], in_=wb_b[:])
            nc.sync.dma_start(out=xh_b[:], in_=xh)
            if use_cc:
                nc.gpsimd.collective_compute(
                    "AllGather", AL.bypass, replica_groups=PAIRS,
                    ins=[xh_b[:].opt()], outs=[xg[:].opt()])
            else:
                for r in range(2):
                    nc.gpsimd.dma_start(
                        out=xg[r * 2 * C:(r + 1) * 2 * C, :], in_=xh_b[:])

            # ---- weight / bias / input loads into SBUF (batched DMAs) ----
            # wg rows: 0:768 wqk [6k,128,768]; 768:1152 wv packed
            # [3kk,128,2,384]; 1152:1536 wproj [3p,128,768]
            wqk_sb = pp.tile([128, KT * C], BF16, tag="wqk", name="wqk")
            nc.sync.dma_start(
                out=wqk_sb.rearrange("p (k c) -> p k c", k=KT),
                in_=wg[0:C, :].rearrange("(k p) c -> p k c", k=KT))
            wv_sb = pp.tile([128, KT * HPC * D], BF16, tag="wv", name="wv")
            nc.sync.dma_start(
                out=wv_sb.rearrange("p (kk two c) -> p kk two c", kk=3, two=2),
                in_=wg[C:C + 384, :].rearrange(
                    "(kk p) (two c) -> p kk two c", kk=3, two=2))
            wp_sb = pp.tile([128, (HPC // 2) * C], BF16, tag="wp", name="wp")
            nc.sync.dma_start(
                out=wp_sb.rearrange("p (q c) -> p q c", q=HPC // 2),
                in_=wg[1152:1536, :].rearrange("(q p) c -> p q c", q=HPC // 2))
            bq_sb = pp.tile([128, 2 * HPC], F32, tag="bq")
            nc.sync.dma_start(out=bq_sb, in_=bqkv)
            bp_sb = pp.tile([128, KT], F32, tag="bp")
            nc.sync.dma_start(out=bp_sb, in_=bproj)
            bv_bc = pp.tile([128, HPC * D], F32, tag="bvbc")
            nc.gpsimd.dma_start(
                out=bv_bc,
                in_=bass.AP(tensor=bv.tensor, offset=0,
                            ap=[[0, 128], [1, HPC * D]]),
            )
            # x_sb[s]: [128, KT*N] bf16, col block k*N+q*512 = chunk k q-half
            x_sb = [pp.tile([128, KT * N], BF16, tag=f"x{s}", name=f"x{s}")
                    for s in range(2)]
            for g in range(2):
                for s in range(2):
                    nc.scalar.dma_start(
                        out=x_sb[s].rearrange("p (k t) -> p k t", k=KT)[
                            :, :, g * 512:(g + 1) * 512],
                        in_=xg[g * 2 * C + s * C:g * 2 * C + (s + 1) * C, :]
                        .rearrange("(k p) t -> p k t", k=KT))
            ones = pp.tile([128, 128], BF16, tag="ones")
            nc.vector.memset(ones, 1.0)

            # ---- phase 1: q/k and v projections, interleaved ----
            # v passes are woven between qk passes so the PE's feeder load
            # (DVE psum drains) averages below the PE rate and the v block
            # no longer delays attention. Head 0/1's q=1 qk passes run
            # right after q=0 so head 0's first two score tiles can be
            # emitted mid-phase — the ACT engine then starts the exp
            # pipeline during the qkv tail.
            do_attn = phase_cut in (None, "attn")
            qt = [pp.tile([128, N], BF16, tag=f"qt{h}", name=f"qt{h}")
                  for h in range(HPC)]
            kt_ = [pp.tile([128, N], BF16, tag=f"kt{h}", name=f"kt{h}")
                   for h in range(HPC)]
            vt = [pp.tile([128, HPC * 128], BF16, tag=f"vt{t}", name=f"vt{t}")
                  for t in range(NK)]
            ps_qk = tc.alloc_tile_pool(name="ps_qk", bufs=2, space="PSUM")
            ps_v = tc.alloc_tile_pool(name="ps_v", bufs=2, space="PSUM")

            def qk_pass(q, s, ft):
                p = ps_qk.tile([128, 512], F32, tag="qkp", name="qkp", bufs=4)
                for k in range(KT):
                    nc.tensor.matmul(
                        p,
                        lhsT=wqk_sb[:, k * C + ft * 128:k * C + (ft + 1) * 128],
                        rhs=x_sb[s][:, k * N + q * 512:k * N + (q + 1) * 512],
                        start=(k == 0), stop=(k == KT - 1))
                pair = qt if ft < HPC // 2 else kt_
                h0 = (ft % (HPC // 2)) * 2
                for hf in range(2):
                    nc.vector.tensor_scalar(
                        out=pair[h0 + hf][s * 64:(s + 1) * 64,
                                          q * 512:(q + 1) * 512],
                        in0=p[hf * 64:(hf + 1) * 64, :],
                        scalar1=bq_sb[hf * 64:(hf + 1) * 64,
                                      2 * ft:2 * ft + 1],
                        scalar2=None, op0=AL.add)

            def v_pass(s, t):
                p = ps_v.tile([128, HPC * D], F32, tag="vp", name="vp",
                              bufs=2)
                for k in range(KT):
                    nc.tensor.matmul(
                        p,
                        lhsT=x_sb[s][:, k * N + t * 128:k * N + (t + 1) * 128],
                        rhs=wv_sb[:, k * HPC * D:(k + 1) * HPC * D],
                        start=(k == 0), stop=(k == KT - 1))
                out3 = vt[t].rearrange(
                    "p (h two d) -> p h two d", two=2, d=D)[:, :, s, :]
                nc.vector.tensor_tensor(
                    out=out3,
                    in0=p.rearrange("p (h d) -> p h d", d=D),
                    in1=bv_bc.rearrange("p (h d) -> p h d", d=D),
                    op=AL.add)

            def scores_tile(h, kb):
                sp = ps_s.tile([128, N], F32, tag=f"sp{kb % 2}",
                               name="sp", bufs=1)
                for q in range(NQ):
                    nc.tensor.matmul(
                        sp[:, q * 512:(q + 1) * 512],
                        lhsT=kt_[h][:, kb * 128:(kb + 1) * 128],
                        rhs=qt[h][:, q * 512:(q + 1) * 512],
                        start=True, stop=True)
                return sp

            for q in range(NQ):
                for s in range(2):
                    for ft in range(HPC):
                        qk_pass(q, s, ft)
            pre_sp = None
            for s in range(2):
                for t in range(NK):
                    v_pass(s, t)
            ps_v.release()
            ps_qk.release()

            if phase_cut == "qkv":
                for h in range(HPC):
                    nc.sync.dma_start(out=yt[h * 128:(h + 1) * 128, 0:512],
                                      in_=qt[h][:, 0:512])
                    nc.sync.dma_start(out=yt[h * 128:(h + 1) * 128, 512:1024],
                                      in_=kt_[h][:, 0:512])
                for t in range(NK):
                    nc.sync.dma_start(
                        out=yt[(t % 6) * 128:(t % 6 + 1) * 128,
                               (t // 6) * 128:(t // 6) * 128 + 128],
                        in_=vt[t][:, 0:128])

            if phase_cut in (None, "attn"):
                # ---- phase 2: attention per head, sT[k, q] orientation ----
                # Softmax denominator: bf16 pairwise add-tree over the 8 exp
                # tiles on DVE, then a single ones-matmul pass per head
                # (2 matmuls instead of 16) — keeps the PE fed with scores/AV
                # work. op2 is double-buffered so head h+1's AV can start
                # while head h is normalized; normalization writes straight
                # into the stream-packed ost layout via partition-shifted DVE.
                ost = [[pp.tile([128, N], BF16, tag=f"ost{s}_{p}",
                                name=f"ost{s}_{p}")
                        for p in range(HPC // 2)] for s in range(2)]
                dsp = tc.alloc_tile_pool(name="dsp", bufs=3)
                ps_s = tc.alloc_tile_pool(name="ps_s", bufs=2, space="PSUM")
                ps_o = tc.alloc_tile_pool(name="ps_o", bufs=2, space="PSUM")
                fin_prev = None
                for h in range(HPC):
                    op2 = ps_o.tile([128, N], F32, tag="op2", name="op2")
                    sp = [None] * NK
                    ex = [None] * NK
                    tr = {}

                    if h == 0 and pre_sp is not None:
                        sp[0], sp[1] = pre_sp
                    else:
                        sp[0] = scores_tile(h, 0)
                        sp[1] = scores_tile(h, 1)
                    if fin_prev is not None:
                        fin_prev()
                        fin_prev = None
                    for kb in range(NK):
                        ex[kb] = ep.tile([128, N], BF16, tag="exp", name="exp")
                        nc.scalar.activation(out=ex[kb], in_=sp[kb], func=AF.Exp)
                        for q in range(NQ):
                            nc.tensor.matmul(
                                op2[:, q * 512:(q + 1) * 512],
                                lhsT=vt[kb][:, h * 128:(h + 1) * 128],
                                rhs=ex[kb][:, q * 512:(q + 1) * 512],
                                start=(kb == 0), stop=(kb == NK - 1))
                        if kb % 2 == 1:
                            t = dsp.tile([128, N], BF16, tag=f"t{kb // 2}",
                                         name="t")
                            nc.vector.tensor_tensor(out=t, in0=ex[kb - 1],
                                                    in1=ex[kb], op=AL.add)
                            tr[kb // 2] = t
                        if kb == 3:
                            ab = dsp.tile([128, N], BF16, tag="ab", name="ab")
                            nc.vector.tensor_tensor(out=ab, in0=tr[0],
                                                    in1=tr[1], op=AL.add)
                        if kb == 7:
                            cd = dsp.tile([128, N], BF16, tag="cd", name="cd")
                            nc.vector.tensor_tensor(out=cd, in0=tr[2],
                                                    in1=tr[3], op=AL.add)
                            es = dsp.tile([128, N], BF16, tag="es", name="es")
                            nc.vector.tensor_tensor(out=es, in0=ab,
                                                    in1=cd, op=AL.add)
                        if kb + 2 < NK:
                            sp[kb + 2] = scores_tile(h, kb + 2)

                    def finalize(h=h, op2=op2, es=es):
                        dp2 = ps_s.tile([128, N], F32, tag="sp0",
                                        name="dp2", bufs=1)
                        for q in range(NQ):
                            nc.tensor.matmul(
                                dp2[:, q * 512:(q + 1) * 512],
                                lhsT=ones,
                                rhs=es[:, q * 512:(q + 1) * 512],
                                start=True, stop=True)
                        rd = rp.tile([128, N], F32, tag="rd", name="rd")
                        nc.vector.reciprocal_approx_fast(out=rd, in_=dp2)
                        for s in range(2):
                            nc.vector.tensor_mul(
                                out=ost[s][h // 2][(h % 2) * 64:
                                                   (h % 2) * 64 + 64, :],
                                in0=op2[s * 64:(s + 1) * 64, :],
                                in1=rd[s * 64:(s + 1) * 64, :])

                    fin_prev = finalize
                fin_prev()
                ps_o.release()
                ps_s.release()
                dsp.release()

                if phase_cut == "attn":
                    for s in range(2):
                        for p_ in range(HPC // 2):
                            nc.sync.dma_start(
                                out=yt[(s * 3 + p_) * 128:
                                       (s * 3 + p_ + 1) * 128, :],
                                in_=ost[s][p_])

            if phase_cut is None:
                # ---- phase 3: projection + bias/2, pair ReduceScatter ----
                ystk = [dp_.tile([2 * C, 512], BF16, tag=f"ystk{q}",
                                 name=f"ystk{q}") for q in range(NQ)]
                yrs = [dp_.tile([C, 512], BF16, tag=f"yrs{q}",
                                name=f"yrs{q}") for q in range(NQ)]
                ps_y = tc.alloc_tile_pool(name="ps_y", bufs=4, space="PSUM")
                NP = HPC // 2
                for q in range(NQ):
                    for cb in range(C // 128):
                        for s in range(2):
                            py = ps_y.tile([128, 512], F32, tag="yp", name="yp")
                            for p in range(NP):
                                nc.tensor.matmul(
                                    py,
                                    lhsT=wp_sb[:, p * C + cb * 128:
                                               p * C + (cb + 1) * 128],
                                    rhs=ost[s][p][:, q * 512:(q + 1) * 512],
                                    start=(p == 0), stop=(p == NP - 1))
                            yb = yp.tile([128, 512], BF16, tag="yb")
                            nc.vector.tensor_scalar(
                                out=yb, in0=py,
                                scalar1=bp_sb[:, cb:cb + 1],
                                scalar2=None, op0=AL.add)
                            nc.sync.dma_start(
                                out=ystk[q][s * C + cb * 128:
                                            s * C + (cb + 1) * 128, :],
                                in_=yb)
                    if use_cc:
                        nc.gpsimd.collective_compute(
                            "ReduceScatter", AL.add, replica_groups=PAIRS,
                            ins=[ystk[q][:].opt()], outs=[yrs[q][:].opt()])
                    else:
                        nc.gpsimd.dma_start(out=yrs[q][:],
                                            in_=ystk[q][0:C, :])
                    nc.sync.dma_start(out=yt[:, q * 512:(q + 1) * 512],
                                      in_=yrs[q][:])
                ps_y.release()

    nc.compile()
    return nc


_cache = threading.Lock()
_nc = None
_runner = None


def _get_program():
    global _nc
    with _cache:
        if _nc is None:
            _nc = build_program()
    return _nc


class _Runner:
    """Compile the 8-core sharded PJRT callable once and reuse it across
    kernel() calls (run_bass_kernel_spmd re-traces jax.jit per call, which
    costs seconds; the NEFF itself is what actually runs)."""

    def __init__(self, nc, n_cores=8):
        install_neuronx_cc_hook()
        self.nc = nc
        self.n_cores = n_cores
        partition_name = (nc.partition_id_tensor.name
                          if nc.partition_id_tensor else None)
        in_names, out_names, out_avals, zero_outs = [], [], [], []
        for alloc in nc.m.functions[0].allocations:
            if not isinstance(alloc, mybir.MemoryLocationSet):
                continue
            name = alloc.memorylocations[0].name
            if alloc.kind == "ExternalInput":
                if name != partition_name:
                    in_names.append(name)
            elif alloc.kind == "ExternalOutput":
                out_names.append(name)
                shape = tuple(alloc.tensor_shape)
                dtype = mybir.dt.np(alloc.dtype)
                out_avals.append(jax.core.ShapedArray(shape, dtype))
                zero_outs.append(
                    np.zeros((n_cores * shape[0], *shape[1:]), dtype))
        self.in_names = in_names
        self.out_names = out_names
        self.out_shapes = [tuple(a.shape) for a in out_avals]
        self.zero_outs = zero_outs
        n_params = len(in_names)
        n_outs = len(out_avals)
        all_in = list(in_names) + list(out_names)
        if partition_name is not None:
            all_in.append(partition_name)

        def _body(*args):
            operands = list(args)
            if partition_name is not None:
                operands.append(partition_id_tensor())
            outs = _bass_exec_p.bind(
                *operands,
                out_avals=tuple(out_avals),
                in_names=tuple(all_in),
                out_names=tuple(out_names),
                lowering_input_output_aliases=(),
                sim_require_finite=True,
                sim_require_nnan=True,
                nc=nc,
            )
            return tuple(outs)

        devices = jax.devices()[:n_cores]
        mesh = Mesh(np.asarray(devices), ("core",))
        self.f = jax.jit(
            shard_map(
                _body, mesh=mesh,
                in_specs=(PartitionSpec("core"),) * (n_params + n_outs),
                out_specs=(PartitionSpec("core"),) * n_outs,
                check_rep=False,
            ),
            keep_unused=True,
        )

    def run(self, in_maps):
        n = self.n_cores
        concat_in = [
            np.concatenate([np.asarray(in_maps[c][name]) for c in range(n)],
                           axis=0)
            for name in self.in_names
        ]
        out_arrs = self.f(*concat_in, *self.zero_outs)
        return [
            {name: np.asarray(out_arrs[i]).reshape(n, *self.out_shapes[i])[c]
             for i, name in enumerate(self.out_names)}
            for c in range(n)
        ]


def _get_runner():
    global _runner
    nc = _get_program()
    with _cache:
        if _runner is None:
            _runner = _Runner(nc)
    return _runner


def _f32_to_bf16(a):
    """Fast round-to-nearest f32->bf16 via integer ops (contiguous input)."""
    u = np.ascontiguousarray(a, np.float32).view(np.uint32)
    return (((u + 0x7FFF) + ((u >> 16) & 1)) >> 16).astype(np.uint16).view(BF)


def _bf16_to_f32(a):
    return (np.asarray(a).view(np.uint16).astype(np.uint32) << 16).view(
        np.float32)


_wprep_cache = {}


def _prep_weights(W_qkv, b_qkv, W_proj, b_proj):
    key = (id(W_qkv), id(b_qkv), id(W_proj), id(b_proj))
    hit = _wprep_cache.get(key)
    if hit is not None and (hit[0] is W_qkv and hit[1] is b_qkv
                            and hit[2] is W_proj and hit[3] is b_proj):
        return hit[4]
    W_qkv = np.asarray(W_qkv, np.float32)
    b_qkv = np.asarray(b_qkv, np.float32)
    W_proj = np.asarray(W_proj, np.float32)
    b_proj = np.asarray(b_proj, np.float32)
    Wq = W_qkv[0:C].reshape(H, D, C) * SCALE
    Wk = W_qkv[C:2 * C].reshape(H, D, C)
    Wv = W_qkv[2 * C:3 * C].reshape(H, D, C)
    bq = b_qkv[0:C].reshape(H, D) * SCALE
    bk = b_qkv[C:2 * C].reshape(H, D)
    bvv = b_qkv[2 * C:3 * C].reshape(H, D)

    per_group = []
    for g in range(2):
        hs = slice(g * HPC, (g + 1) * HPC)
        # wqk rows 0:768: [6k,128, 384 q | 384 k] flattened
        wqk_cols = np.concatenate(
            [Wq[hs].reshape(HPC * D, C).T, Wk[hs].reshape(HPC * D, C).T],
            axis=1)                                        # [C, 768]
        # wv rows: [768, 384] -> [3, 2, 128, 384] -> [3, 128, 2, 384] -> [384, 768]
        wv_cols = Wv[hs].reshape(HPC * D, C).T             # [C, 384]
        wv_pack = wv_cols.reshape(3, 2, 128, 384).transpose(0, 2, 1, 3) \
            .reshape(384, C)
        # wproj rows: [3, 128, 768]
        wproj = np.empty((HPC // 2, 128, C), np.float32)
        for p in range(HPC // 2):
            gh = g * HPC + 2 * p
            wproj[p, 0:64] = W_proj[:, gh * D:(gh + 1) * D].T
            wproj[p, 64:128] = W_proj[:, (gh + 1) * D:(gh + 2) * D].T
        blob = _f32_to_bf16(np.concatenate(
            [wqk_cols, wv_pack, wproj.reshape(384, C)], axis=0))  # [1536, 768]
        bqkv_sb = np.empty((128, 2 * HPC), np.float32)
        for ft in range(HPC // 2):
            bqkv_sb[0:64, 2 * ft] = bq[g * HPC + 2 * ft]
            bqkv_sb[64:128, 2 * ft] = bq[g * HPC + 2 * ft + 1]
            bqkv_sb[0:64, 2 * (HPC // 2 + ft)] = bk[g * HPC + 2 * ft]
            bqkv_sb[64:128, 2 * (HPC // 2 + ft)] = bk[g * HPC + 2 * ft + 1]
        bv_row = np.ascontiguousarray(bvv[hs].reshape(1, HPC * D))
        per_group.append((blob, bqkv_sb, bv_row))
    bproj_sb = np.ascontiguousarray(
        b_proj.reshape(KT, 128).T * 0.5)                   # [128, 6]
    prep = (per_group, bproj_sb)
    _wprep_cache.clear()
    _wprep_cache[key] = (W_qkv, b_qkv, W_proj, b_proj, prep)
    return prep


def make_in_maps(x1, x2, W_qkv, b_qkv, W_proj, b_proj):
    """Host-side shard prep. Core c -> (batch c//2, head-group c%2)."""
    per_group, bproj_sb = _prep_weights(W_qkv, b_qkv, W_proj, b_proj)
    x1 = np.asarray(x1, np.float32)
    x2 = np.asarray(x2, np.float32)
    # xT per batch in bf16 (fast int-trick cast of the transposed view is
    # slow; transpose small column-blocks instead)
    x1tb = [_f32_to_bf16(np.ascontiguousarray(x1[b].T)) for b in range(B)]
    x2tb = [_f32_to_bf16(np.ascontiguousarray(x2[b].T)) for b in range(B)]

    in_maps = []
    for c in range(8):
        b, g = divmod(c, 2)
        blob, bqkv_sb, bv_row = per_group[g]
        xh = np.concatenate([x1tb[b][:, g * 512:(g + 1) * 512],
                             x2tb[b][:, g * 512:(g + 1) * 512]], axis=0)
        r = c // 2  # rank in quad
        in_maps.append({
            "xh": np.ascontiguousarray(xh),
            "wblob": np.ascontiguousarray(blob[r * 384:(r + 1) * 384]),
            "bqkv": bqkv_sb,
            "bv": bv_row,
            "bproj": bproj_sb,
        })
    return in_maps


def combine_outputs(results):
    y1 = np.empty((B, N, C), np.float32)
    y2 = np.empty((B, N, C), np.float32)
    for b in range(B):
        y1[b] = _bf16_to_f32(results[2 * b]["yt"]).T
        y2[b] = _bf16_to_f32(results[2 * b + 1]["yt"]).T
    return y1, y2


def kernel(x1, x2, W_qkv, b_qkv, W_proj, b_proj):
    in_maps = make_in_maps(x1, x2, W_qkv, b_qkv, W_proj, b_proj)
    try:
        results = _get_runner().run(in_maps)
    except Exception:
        # robust fallback: the one-shot path run_bass_kernel_spmd uses
        nc = _get_program()
        results = run_bass_kernel_spmd(
            nc, in_maps, core_ids=list(range(8))).results
    return combine_outputs(results)



# revision 7
# speedup vs baseline: 1.3353x; 1.3353x over previous
"""Dual-stream attention (nn_Attention2) on 8 TRN2 NeuronCores, v3.

Problem: B=4, N=1024, C=768, H=12, D=64.
  qkv_s = x_s @ W_qkv.T + b_qkv          (s = 1,2; shared weights)
  attn  = softmax(q1k1/sqrt(D) + q2k2/sqrt(D))   (one shared softmax)
  o_s   = attn @ v_s;  y_s = o_s @ W_proj.T + b_proj

Sharding: 8 cores = 4 batches x 2 head-groups (6 heads each). v3 drops
ALL on-device collectives: every core receives its full inputs (x for
its batch, weights for its head group) pre-staged in device DRAM in
SBUF-ready [128, free] layout, so SBUF loads are single contiguous DMAs
and the PE starts ~4us in (v2 burned ~30us on staging copies +
AllGathers). Each core emits its PARTIAL projection output (both
streams, bf16); the host adds the pair partials (the "unshard" step),
which replaces v2's tail ReduceScatter.

Bias handling: q/k biases are applied for free on the mandatory
psum->SBUF drains (ACT Identity-with-bias / DVE tensor_scalar).  v and
proj biases fold into a single host-side constant vector: softmax rows
sum to 1, so  o = attn@(v + bv) = attn@v + bv  and the bias lands in
y as  W_proj @ bv + b_proj, added on the host.

Compute phases (per core, stream-stacked [s1 64 | s2 64] q/k tiles so
combined scores are one matmul chain; attention in sT[k,q] orientation;
ones-matmul gives the softmax denominator broadcast across partitions):
  1. qkv: 12 q/k passes ([128,1024] psum, drains alternate DVE/ACT),
     16 v passes ([128,384] psum, plain-copy drains).
  2. attention per head: scores 2 tiles ahead, exp on ACT (bf16 out),
     AV accumulated into a single [128,1024] psum, bf16 pairwise
     add-tree on DVE + ones-matmul denominator in its OWN psum tag
     (v2 aliased it with the scores ring, serializing ~2us/head),
     reciprocal+normalize on DVE, finalize deferred one head.
  3. projection: per (q,cb,s) 3-matmul chains, Copy drains on ACT/DVE,
     partial y DMAed straight out (no collective, no device bias).
bf16 matmuls, f32 PSUM.
"""

import contextlib
import threading

import numpy as np
import ml_dtypes
import jax
from jax.sharding import Mesh, PartitionSpec
try:
    from jax.experimental.shard_map import shard_map
except ImportError:
    from jax.sharding import shard_map

import concourse.bass as bass
import concourse.tile as tile
from concourse import bacc, mybir
from concourse.bass_utils import run_bass_kernel_spmd
from concourse.bass2jax import (
    install_neuronx_cc_hook,
    partition_id_tensor,
    _bass_exec_p,
)

F32 = mybir.dt.float32
BF16 = mybir.dt.bfloat16
AL = mybir.AluOpType
AF = mybir.ActivationFunctionType

B, N, C, H = 4, 1024, 768, 12
D = C // H              # 64
HPC = 6                 # heads per core
KT = C // 128           # 6 contraction tiles over C
NQ = N // 512           # 2 q-halves
NK = N // 128           # 8 k-blocks
SCALE = float(D) ** -0.5
BF = ml_dtypes.bfloat16


def build_program(loop_reps=0, phase_cut=None, use_cc=True):
    """use_cc is accepted for test.py compat; v3 has no collectives, so
    the timing build and the graded build are the same program."""
    del use_cc
    nc = bacc.Bacc("TRN2", target_bir_lowering=False, debug=False)

    # inputs pre-staged in DRAM in SBUF layout ([128, free], bf16)
    wqk_d = nc.dram_tensor("wqk", [128, KT * C], BF16, kind="ExternalInput").ap()
    wv_d = nc.dram_tensor("wv", [128, KT * HPC * D], BF16,
                          kind="ExternalInput").ap()
    wp_d = nc.dram_tensor("wp", [128, (HPC // 2) * C], BF16,
                          kind="ExternalInput").ap()
    x_d = [nc.dram_tensor(f"x{s}", [128, KT * N], BF16,
                          kind="ExternalInput").ap() for s in range(2)]
    bq_d = nc.dram_tensor("bq", [128, HPC], F32, kind="ExternalInput").ap()
    yp = nc.dram_tensor("yp", [2 * C, N], BF16, kind="ExternalOutput").ap()

    with tile.TileContext(nc) as tc:
        with (
            tc.tile_pool(name="persist", bufs=1) as pp,
            tc.tile_pool(name="expp", bufs=8) as ep,
            tc.tile_pool(name="rdp", bufs=2) as rp,
            tc.tile_pool(name="ybp", bufs=4) as yp_pool,
            tc.For_i(0, loop_reps, 1) if loop_reps else contextlib.nullcontext(),
        ):
            # ---- SBUF loads: contiguous DMAs straight from DRAM inputs ----
            wqk_sb = pp.tile([128, KT * C], BF16, tag="wqk", name="wqk")
            x_sb = [pp.tile([128, KT * N], BF16, tag=f"x{s}", name=f"x{s}")
                    for s in range(2)]
            wv_sb = pp.tile([128, KT * HPC * D], BF16, tag="wv", name="wv")
            wp_sb = pp.tile([128, (HPC // 2) * C], BF16, tag="wp", name="wp")
            bq_sb = pp.tile([128, HPC], F32, tag="bq")
            # split the loads into chunks across issue queues so the first
            # qk_pass can start ~1us in instead of waiting for monolithic
            # transfers on one serial queue.
            for k in range(KT):
                nc.sync.dma_start(out=wqk_sb[:, k * C:(k + 1) * C],
                                  in_=wqk_d[:, k * C:(k + 1) * C])
            half = KT * 512
            for q in range(NQ):
                for s in range(2):
                    eng = nc.scalar if s == 0 else nc.vector
                    eng.dma_start(
                        out=x_sb[s][:, q * half:(q + 1) * half],
                        in_=x_d[s][:, q * half:(q + 1) * half])
            nc.gpsimd.dma_start(out=bq_sb, in_=bq_d)
            nc.gpsimd.dma_start(out=wv_sb, in_=wv_d)
            nc.gpsimd.dma_start(out=wp_sb, in_=wp_d)
            ones = pp.tile([128, 128], BF16, tag="ones")
            nc.vector.memset(ones, 1.0)

            # x_sb column layout: q*(KT*512) + k*512 + t  (t in 0..511)
            def xcol(s, q, k, t0, n):
                base = q * (KT * 512) + k * 512 + t0
                return x_sb[s][:, base:base + n]

            # ---- phase 1: q/k then v projections ----
            qt = [pp.tile([128, N], BF16, tag=f"qt{h}", name=f"qt{h}")
                  for h in range(HPC)]
            kt_ = [pp.tile([128, N], BF16, tag=f"kt{h}", name=f"kt{h}")
                   for h in range(HPC)]
            vt = [pp.tile([128, HPC * 128], BF16, tag=f"vt{t}", name=f"vt{t}")
                  for t in range(NK)]

            ps_qk = tc.alloc_tile_pool(name="ps_qk", bufs=3, space="PSUM")
            ps_v = tc.alloc_tile_pool(name="ps_v", bufs=2, space="PSUM")

            def qk_pass(ft, s):
                # both q-halves in one [128,1024] psum; 12 matmuls
                p = ps_qk.tile([128, N], F32, tag="qkp", name="qkp")
                for q in range(NQ):
                    for k in range(KT):
                        nc.tensor.matmul(
                            p[:, q * 512:(q + 1) * 512],
                            lhsT=wqk_sb[:, k * C + ft * 128:
                                        k * C + (ft + 1) * 128],
                            rhs=xcol(s, q, k, 0, 512),
                            start=(k == 0), stop=(k == KT - 1))
                pair = qt if ft < HPC // 2 else kt_
                h0 = (ft % (HPC // 2)) * 2
                # one drain per head-half: hf=0 on DVE, hf=1 on ACT
                nc.vector.tensor_scalar(
                    out=pair[h0][s * 64:(s + 1) * 64, :],
                    in0=p[0:64, :],
                    scalar1=bq_sb[0:64, ft:ft + 1],
                    scalar2=None, op0=AL.add)
                nc.scalar.activation(
                    out=pair[h0 + 1][s * 64:(s + 1) * 64, :],
                    in_=p[64:128, :],
                    func=AF.Identity,
                    bias=bq_sb[64:128, ft:ft + 1])

            def v_pass(s, t):
                p = ps_v.tile([128, HPC * D], F32, tag="vp", name="vp")
                q = t // 4
                t0 = t * 128 - q * 512
                for k in range(KT):
                    nc.tensor.matmul(
                        p,
                        lhsT=xcol(s, q, k, t0, 128),
                        rhs=wv_sb[:, k * HPC * D:(k + 1) * HPC * D],
                        start=(k == 0), stop=(k == KT - 1))
                out3 = vt[t].rearrange(
                    "p (h two d) -> p h two d", two=2, d=D)[:, :, s, :]
                src = p.rearrange("p (h d) -> p h d", d=D)
                if (s + t) % 2 == 0:
                    nc.vector.tensor_copy(out=out3, in_=src)
                else:
                    nc.scalar.activation(out=out3, in_=src, func=AF.Copy)

            for ft in range(HPC):
                for s in range(2):
                    qk_pass(ft, s)
            for t in range(NK):
                for s in range(2):
                    v_pass(s, t)
            ps_v.release()
            ps_qk.release()

            if phase_cut == "qkv":
                for h in range(HPC):
                    nc.sync.dma_start(out=yp[h * 128:(h + 1) * 128, 0:512],
                                      in_=qt[h][:, 0:512])
                    nc.sync.dma_start(out=yp[h * 128:(h + 1) * 128, 512:1024],
                                      in_=kt_[h][:, 0:512])
                for t in range(NK):
                    nc.sync.dma_start(
                        out=yp[C + (t % 6) * 128:C + (t % 6 + 1) * 128,
                               (t // 6) * 128:(t // 6) * 128 + 128],
                        in_=vt[t][:, 0:128])

            if phase_cut in (None, "attn"):
                # ---- phase 2: attention per head, sT[k, q] orientation ----
                ost = [[pp.tile([128, N], BF16, tag=f"ost{s}_{p}",
                                name=f"ost{s}_{p}")
                        for p in range(HPC // 2)] for s in range(2)]
                dsp = tc.alloc_tile_pool(name="dsp", bufs=2)
                ps_s = tc.alloc_tile_pool(name="ps_s", bufs=2, space="PSUM")
                ps_o = tc.alloc_tile_pool(name="ps_o", bufs=1, space="PSUM")
                fin_prev = None
                for h in range(HPC):
                    op2 = ps_o.tile([128, N], F32, tag="op2", name="op2")
                    sp = [None] * NK
                    ex = [None] * NK
                    tr = {}

                    def scores_tile(h, kb):
                        spt = ps_s.tile([128, N], F32, tag=f"sp{kb % 2}",
                                        name="sp", bufs=1)
                        for q in range(NQ):
                            nc.tensor.matmul(
                                spt[:, q * 512:(q + 1) * 512],
                                lhsT=kt_[h][:, kb * 128:(kb + 1) * 128],
                                rhs=qt[h][:, q * 512:(q + 1) * 512],
                                start=True, stop=True)
                        return spt

                    sp[0] = scores_tile(h, 0)
                    sp[1] = scores_tile(h, 1)
                    if fin_prev is not None:
                        fin_prev()
                        fin_prev = None
                    for kb in range(NK):
                        ex[kb] = ep.tile([128, N], BF16, tag="exp", name="exp")
                        nc.scalar.activation(out=ex[kb], in_=sp[kb], func=AF.Exp)
                        for q in range(NQ):
                            nc.tensor.matmul(
                                op2[:, q * 512:(q + 1) * 512],
                                lhsT=vt[kb][:, h * 128:(h + 1) * 128],
                                rhs=ex[kb][:, q * 512:(q + 1) * 512],
                                start=(kb == 0), stop=(kb == NK - 1))
                        if kb % 2 == 1:
                            t = dsp.tile([128, N], BF16, tag=f"t{kb // 2}",
                                         name="t")
                            nc.vector.tensor_tensor(out=t, in0=ex[kb - 1],
                                                    in1=ex[kb], op=AL.add)
                            tr[kb // 2] = t
                        if kb == 3:
                            ab = dsp.tile([128, N], BF16, tag="ab", name="ab")
                            nc.vector.tensor_tensor(out=ab, in0=tr[0],
                                                    in1=tr[1], op=AL.add)
                        if kb == 7:
                            cd = dsp.tile([128, N], BF16, tag="cd", name="cd")
                            nc.vector.tensor_tensor(out=cd, in0=tr[2],
                                                    in1=tr[3], op=AL.add)
                            es = dsp.tile([128, N], BF16, tag="es", name="es")
                            nc.vector.tensor_tensor(out=es, in0=ab,
                                                    in1=cd, op=AL.add)
                        if kb + 2 < NK:
                            sp[kb + 2] = scores_tile(h, kb + 2)

                    def finalize(h=h, op2=op2, es=es):
                        # q-half-split chain halves the denominator->
                        # normalize latency that gates op2 reuse
                        dp2 = ps_s.tile([128, N], F32, tag="dp",
                                        name="dp2", bufs=1)
                        rd = rp.tile([128, N], F32, tag="rd", name="rd")
                        for q in range(NQ):
                            ql = slice(q * 512, (q + 1) * 512)
                            nc.tensor.matmul(
                                dp2[:, ql], lhsT=ones, rhs=es[:, ql],
                                start=True, stop=True)
                            nc.vector.reciprocal_approx_fast(
                                out=rd[:, ql], in_=dp2[:, ql])
                            for s in range(2):
                                nc.vector.tensor_mul(
                                    out=ost[s][h // 2][(h % 2) * 64:
                                                       (h % 2) * 64 + 64, ql],
                                    in0=op2[s * 64:(s + 1) * 64, ql],
                                    in1=rd[s * 64:(s + 1) * 64, ql])

                    fin_prev = finalize
                fin_prev()
                ps_o.release()
                ps_s.release()
                dsp.release()

                if phase_cut == "attn":
                    for s in range(2):
                        for p_ in range(HPC // 2):
                            nc.sync.dma_start(
                                out=yp[(s * 3 + p_) * 128:
                                       (s * 3 + p_ + 1) * 128, :],
                                in_=ost[s][p_])

            if phase_cut is None:
                # ---- phase 3: projection; partial y straight to DRAM ----
                ps_y = tc.alloc_tile_pool(name="ps_y", bufs=4, space="PSUM")
                NP = HPC // 2
                for q in range(NQ):
                    for cb in range(C // 128):
                        for s in range(2):
                            py = ps_y.tile([128, 512], F32, tag="yp", name="yp")
                            for p in range(NP):
                                nc.tensor.matmul(
                                    py,
                                    lhsT=wp_sb[:, p * C + cb * 128:
                                               p * C + (cb + 1) * 128],
                                    rhs=ost[s][p][:, q * 512:(q + 1) * 512],
                                    start=(p == 0), stop=(p == NP - 1))
                            yb = yp_pool.tile([128, 512], BF16, tag="yb")
                            if (cb + s) % 2 == 0:
                                nc.scalar.activation(out=yb, in_=py,
                                                     func=AF.Copy)
                            else:
                                nc.vector.tensor_copy(out=yb, in_=py)
                            nc.sync.dma_start(
                                out=yp[s * C + cb * 128:s * C + (cb + 1) * 128,
                                       q * 512:(q + 1) * 512],
                                in_=yb)
                ps_y.release()

    nc.compile()
    return nc


_cache = threading.Lock()
_nc = None
_runner = None


def _get_program():
    global _nc
    with _cache:
        if _nc is None:
            _nc = build_program()
    return _nc


class _Runner:
    """Compile the 8-core sharded PJRT callable once and reuse it across
    kernel() calls (run_bass_kernel_spmd re-traces jax.jit per call, which
    costs seconds; the NEFF itself is what actually runs)."""

    def __init__(self, nc, n_cores=8):
        install_neuronx_cc_hook()
        self.nc = nc
        self.n_cores = n_cores
        partition_name = (nc.partition_id_tensor.name
                          if nc.partition_id_tensor else None)
        in_names, out_names, out_avals, zero_outs = [], [], [], []
        for alloc in nc.m.functions[0].allocations:
            if not isinstance(alloc, mybir.MemoryLocationSet):
                continue
            name = alloc.memorylocations[0].name
            if alloc.kind == "ExternalInput":
                if name != partition_name:
                    in_names.append(name)
            elif alloc.kind == "ExternalOutput":
                out_names.append(name)
                shape = tuple(alloc.tensor_shape)
                dtype = mybir.dt.np(alloc.dtype)
                out_avals.append(jax.core.ShapedArray(shape, dtype))
                zero_outs.append(
                    np.zeros((n_cores * shape[0], *shape[1:]), dtype))
        self.in_names = in_names
        self.out_names = out_names
        self.out_shapes = [tuple(a.shape) for a in out_avals]
        self.zero_outs = zero_outs
        n_params = len(in_names)
        n_outs = len(out_avals)
        all_in = list(in_names) + list(out_names)
        if partition_name is not None:
            all_in.append(partition_name)

        def _body(*args):
            operands = list(args)
            if partition_name is not None:
                operands.append(partition_id_tensor())
            outs = _bass_exec_p.bind(
                *operands,
                out_avals=tuple(out_avals),
                in_names=tuple(all_in),
                out_names=tuple(out_names),
                lowering_input_output_aliases=(),
                sim_require_finite=True,
                sim_require_nnan=True,
                nc=nc,
            )
            return tuple(outs)

        devices = jax.devices()[:n_cores]
        mesh = Mesh(np.asarray(devices), ("core",))
        self.f = jax.jit(
            shard_map(
                _body, mesh=mesh,
                in_specs=(PartitionSpec("core"),) * (n_params + n_outs),
                out_specs=(PartitionSpec("core"),) * n_outs,
                check_rep=False,
            ),
            keep_unused=True,
        )

    def run(self, in_maps):
        n = self.n_cores
        concat_in = [
            np.concatenate([np.asarray(in_maps[c][name]) for c in range(n)],
                           axis=0)
            for name in self.in_names
        ]
        out_arrs = self.f(*concat_in, *self.zero_outs)
        return [
            {name: np.asarray(out_arrs[i]).reshape(n, *self.out_shapes[i])[c]
             for i, name in enumerate(self.out_names)}
            for c in range(n)
        ]


def _get_runner():
    global _runner
    nc = _get_program()
    with _cache:
        if _runner is None:
            _runner = _Runner(nc)
    return _runner


def _f32_to_bf16(a):
    """Fast round-to-nearest f32->bf16 via integer ops (contiguous input)."""
    u = np.ascontiguousarray(a, np.float32).view(np.uint32)
    return (((u + 0x7FFF) + ((u >> 16) & 1)) >> 16).astype(np.uint16).view(BF)


def _bf16_to_f32(a):
    return (np.asarray(a).view(np.uint16).astype(np.uint32) << 16).view(
        np.float32)


_wprep_cache = {}


def _prep_weights(W_qkv, b_qkv, W_proj, b_proj):
    key = (id(W_qkv), id(b_qkv), id(W_proj), id(b_proj))
    hit = _wprep_cache.get(key)
    if hit is not None and (hit[0] is W_qkv and hit[1] is b_qkv
                            and hit[2] is W_proj and hit[3] is b_proj):
        return hit[4]
    W_qkv = np.asarray(W_qkv, np.float32)
    b_qkv = np.asarray(b_qkv, np.float32)
    W_proj = np.asarray(W_proj, np.float32)
    b_proj = np.asarray(b_proj, np.float32)
    Wq = W_qkv[0:C].reshape(H, D, C) * SCALE
    Wk = W_qkv[C:2 * C].reshape(H, D, C)
    Wv = W_qkv[2 * C:3 * C].reshape(H, D, C)
    bq = b_qkv[0:C].reshape(H, D) * SCALE
    bk = b_qkv[C:2 * C].reshape(H, D)
    bvv = b_qkv[2 * C:3 * C]

    per_group = []
    for g in range(2):
        hs = slice(g * HPC, (g + 1) * HPC)
        # wqk_sb[p, k*768 + ft*128 + hf*64 + d] =
        #   (Wq | Wk)[g*6 + 2*(ft%3) + hf, d, k*128 + p]
        wqk_cols = np.concatenate(
            [Wq[hs].reshape(HPC * D, C).T, Wk[hs].reshape(HPC * D, C).T],
            axis=1)                                        # [C, 768]
        wqk_sb = (wqk_cols.reshape(KT, 128, 2 * HPC * D)
                  .transpose(1, 0, 2).reshape(128, KT * C))
        # wv_sb[p, k*384 + h*64 + d] = Wv[g*6+h, d, k*128+p]
        wv_cols = Wv[hs].reshape(HPC * D, C).T             # [C, 384]
        wv_sb = (wv_cols.reshape(KT, 128, HPC * D)
                 .transpose(1, 0, 2).reshape(128, KT * HPC * D))
        # wp_sb[p, pq*768 + c] = W_proj[c, (g*6+2*pq)*64 + p]
        wproj = np.empty((HPC // 2, 128, C), np.float32)
        for p in range(HPC // 2):
            gh = g * HPC + 2 * p
            wproj[p, 0:64] = W_proj[:, gh * D:(gh + 1) * D].T
            wproj[p, 64:128] = W_proj[:, (gh + 1) * D:(gh + 2) * D].T
        wp_sb = wproj.reshape((HPC // 2) * 128, C).reshape(
            HPC // 2, 128, C).transpose(1, 0, 2).reshape(128, (HPC // 2) * C)
        # bq_sb[hf*64 + d, ft] = (bq | bk)[g*6 + 2*(ft%3) + hf, d]
        bq_sb = np.empty((128, HPC), np.float32)
        for ft in range(HPC):
            src = bq if ft < 3 else bk
            h0 = g * HPC + 2 * (ft % 3)
            bq_sb[0:64, ft] = src[h0]
            bq_sb[64:128, ft] = src[h0 + 1]
        per_group.append((
            np.ascontiguousarray(_f32_to_bf16(wqk_sb)),
            np.ascontiguousarray(_f32_to_bf16(wv_sb)),
            np.ascontiguousarray(_f32_to_bf16(wp_sb)),
            np.ascontiguousarray(bq_sb),
        ))
    # host-side constant: y += b_proj + W_proj @ b_v  (softmax rows sum to 1)
    y_const = b_proj + W_proj @ bvv                       # [C] f32
    prep = (per_group, y_const)
    _wprep_cache.clear()
    _wprep_cache[key] = (W_qkv, b_qkv, W_proj, b_proj, prep)
    return prep


def make_in_maps(x1, x2, W_qkv, b_qkv, W_proj, b_proj):
    """Host-side shard prep. Core c -> (batch c//2, head-group c%2)."""
    per_group, _ = _prep_weights(W_qkv, b_qkv, W_proj, b_proj)
    x1 = np.asarray(x1, np.float32)
    x2 = np.asarray(x2, np.float32)
    # x_sb[p, q*3072 + k*512 + t] = xT[k*128+p, q*512+t], bf16
    xs = []
    for x in (x1, x2):
        per_b = []
        for b in range(B):
            xt = _f32_to_bf16(np.ascontiguousarray(x[b].T))   # [768, 1024]
            per_b.append(np.ascontiguousarray(
                xt.reshape(KT, 128, NQ, 512).transpose(1, 2, 0, 3)
                .reshape(128, KT * N)))
            # note axis order (p, q, k, t): col = q*(KT*512) + k*512 + t
        xs.append(per_b)

    in_maps = []
    for c in range(8):
        b, g = divmod(c, 2)
        wqk_sb, wv_sb, wp_sb, bq_sb = per_group[g]
        in_maps.append({
            "wqk": wqk_sb,
            "wv": wv_sb,
            "wp": wp_sb,
            "x0": xs[0][b],
            "x1": xs[1][b],
            "bq": bq_sb,
        })
    return in_maps


def combine_outputs(results, y_const):
    y1 = np.empty((B, N, C), np.float32)
    y2 = np.empty((B, N, C), np.float32)
    for b in range(B):
        p0 = _bf16_to_f32(results[2 * b]["yp"])
        p1 = _bf16_to_f32(results[2 * b + 1]["yp"])
        ysum = p0 + p1                                     # [2C, N]
        y1[b] = ysum[0:C].T + y_const
        y2[b] = ysum[C:2 * C].T + y_const
    return y1, y2


def kernel(x1, x2, W_qkv, b_qkv, W_proj, b_proj):
    in_maps = make_in_maps(x1, x2, W_qkv, b_qkv, W_proj, b_proj)
    _, y_const = _prep_weights(W_qkv, b_qkv, W_proj, b_proj)
    try:
        results = _get_runner().run(in_maps)
    except Exception:
        # robust fallback: the one-shot path run_bass_kernel_spmd uses
        nc = _get_program()
        results = run_bass_kernel_spmd(
            nc, in_maps, core_ids=list(range(8))).results
    return combine_outputs(results, y_const)
